# revision 22
# baseline (speedup 1.0000x reference)
"""Sliding-window causal attention (B=2,T=2048,C=1024,H=16,D=64,W=256) on 8 trn2 cores.

Sharding: core c = (batch b = c//4, head-group g = c%4 of 4 heads).
Each core computes q/k/v projections for its 4 heads on its batch, windowed
attention per head, and a partial output projection (its 256 channels of the
contraction); host sums the 4 partials per batch.

v2 pipeline (vs v1 baseline):
  - q/k (head-pair 0) projections accumulate kt-major so matmuls track the
    streaming xT DMA instead of waiting for the full 4MB.
  - v-projection is interleaved with pair-0 attention; pair-1 qk projection
    is interleaved with pair-1 scores; out-projection streams behind pair-1
    PV normalization.  PE never idles long enough to re-throttle (HAM).
  - softmax normalization: ACT stages the ones-row denominator to SBUF
    (custom-DVE recip misreads PSUM on HW), DVE reciprocal, partition-
    broadcast via DRAM bounce DMA, one DVE multiply, all deferred half a
    block to hide the latency.
  - PSUM (8 banks): big 1x[128,1024] (scores, 1-deep) | pv 4x[128,512]
    (k-accs, PV+broadcast) | k3 2x[128,512] (warmup, q-accs, v, m1, outproj).
"""

import os
import sys

sys.path.insert(0, "/opt/trn_rl_repo")

import numpy as np
import ml_dtypes

import concourse.bass as bass
import concourse.tile as tile
from concourse import bacc
from concourse import mybir
from concourse.bass import ds, ts

BF16 = ml_dtypes.bfloat16

B, T, C = 2, 2048, 1024
H, W, D = 16, 256, 64
HPC = 4          # heads per core
CL = HPC * D     # 256 local channels per core
NKT = C // 128   # 8 contraction tiles for projections
NT = T // 128    # 16 token tiles
SCALE = 0.125    # 1/sqrt(D)
F32 = mybir.dt.float32
BF = mybir.dt.bfloat16
QB = 512         # normalization granularity along q (4 query tiles)
NBLK = T // QB   # 4 norm blocks
# use an ACT stage copy for the denominator instead of DVE-recip from PSUM
STAGE_RECIP = os.environ.get("ATTN_STAGE_RECIP", "1") == "1"


def build_program():
    nc = bacc.Bacc("TRN2", target_bir_lowering=False, debug=False)

    xT_d = nc.dram_tensor("xTt", [128, NKT * T], BF, kind="ExternalInput")
    w_d = nc.dram_tensor("wt", [128, 3 * NKT * CL], BF, kind="ExternalInput")
    wo_d = nc.dram_tensor("wot", [128, 2 * C], BF, kind="ExternalInput")
    mask_d = nc.dram_tensor("maskt", [128, 512], BF, kind="ExternalInput")
    y_d = nc.dram_tensor("y", [T, C], BF, kind="ExternalOutput")

    with tile.TileContext(nc) as tc:
        with (
            tc.tile_pool(name="const", bufs=1) as constp,
            tc.tile_pool(name="acts", bufs=1) as actsp,
            tc.tile_pool(name="epool", bufs=18) as ep,
            tc.tile_pool(name="small", bufs=4) as smallp,
            tc.tile_pool(name="ysb", bufs=4) as yp,
            tc.tile_pool(name="psA", bufs=1, space="PSUM") as psA,
            tc.tile_pool(name="psB", bufs=1, space="PSUM") as psB,
            tc.tile_pool(name="dramp", bufs=4, space="DRAM") as dramp,
        ):
            # ---- DMA order: mask (warmup), wq+wk (kt-major proj), xT in
            # kt-chunks (512KB each), then wv, wo ----
            mask_all = constp.tile([128, 512], BF, tag="maskall", name="mask_all")
            nc.sync.dma_start(mask_all[:], mask_d[:])
            w_all = constp.tile([128, 3 * NKT * CL], BF, tag="wall", name="w_all")
            xT_all = constp.tile([128, NKT * T], BF, tag="xTall", name="xT_all")
            wo_all = constp.tile([128, 2 * C], BF, tag="woall", name="wo_all")

            def w_load(wi):
                nc.sync.dma_start(
                    w_all[:, ds(wi * NKT * CL, NKT * CL)],
                    w_d[:, ds(wi * NKT * CL, NKT * CL)],
                )

            def x_load(kt):
                nc.sync.dma_start(xT_all[:, ds(kt * T, T)], xT_d[:, ds(kt * T, T)])

            x_load(0)
            w_load(0)          # wq
            w_load(1)          # wk
            for kt in range(1, 7):
                x_load(kt)
            w_load(2)          # wv
            x_load(7)
            nc.sync.dma_start(wo_all[:], wo_d[:])

            xT_sb = [xT_all[:, ds(i * T, T)] for i in range(NKT)]
            wq_sb = [w_all[:, ds((0 * NKT + i) * CL, CL)] for i in range(NKT)]
            wk_sb = [w_all[:, ds((1 * NKT + i) * CL, CL)] for i in range(NKT)]
            wv_sb = [w_all[:, ds((2 * NKT + i) * CL, CL)] for i in range(NKT)]
            wo_sb = [wo_all[:, ds(j * C, C)] for j in range(2)]
            maskf_v = mask_all.rearrange("p (b s x) -> p b s x", b=2, s=2)
            maskd_v = maskf_v[:, :, 0, :]

            ones_sb = constp.tile([1, 64], BF, tag="ones", name="ones_sb")
            nc.gpsimd.memset(ones_sb[:], 1.0)

            # persistent activations
            qT_sb = [actsp.tile([128, T], BF, tag=f"qT{m}", name=f"qT{m}") for m in range(2)]
            kT_sb = [actsp.tile([128, T], BF, tag=f"kT{m}", name=f"kT{m}") for m in range(2)]
            aT_sb = [actsp.tile([128, T], BF, tag=f"aT{m}", name=f"aT{m}") for m in range(2)]
            # v per token-tile: [v_h0(64)|1|v_h1(64)|1|...] = 260 cols
            v_sb = [actsp.tile([128, 4 * 65], BF, tag=f"v{t}", name=f"v{t}") for t in range(NT)]
            for t in range(NT):
                vv = v_sb[t].rearrange("p (h c) -> p h c", h=4)
                nc.gpsimd.memset(vv[:, :, 64:65], 1.0)

            # ---- PE warmup during initial DMA (HAM to 2.4GHz) ----
            n_warm = int(os.environ.get("ATTN_WARMUP_MMS", "24"))
            if n_warm:
                wps = psB.tile([128, 512], F32, tag="k3", bufs=2, name="ps_warm")
                for _ in range(n_warm):
                    nc.tensor.matmul(
                        wps[:], lhsT=mask_all[:, 0:128], rhs=mask_all[:],
                        start=True, stop=True,
                    )

            # ================= phase A: q/k head-pair 0, kt-major =================
            # q chunks 0-1 in the big slot, 2-3 in k3 slots; k chunks in pv slots
            qa = psA.tile([128, 1024], F32, tag="big", name="qa")
            qk3 = [psB.tile([128, 512], F32, tag="k3", bufs=2, name=f"qk3{i}") for i in range(2)]
            kpv = [psB.tile([128, 512], F32, tag="pv", bufs=4, name=f"kpv{i}") for i in range(4)]

            def q_mms(kt):
                st = dict(start=(kt == 0), stop=(kt == NKT - 1))
                for n in range(2):
                    nc.tensor.matmul(
                        qa[:, ts(n, 512)], lhsT=wq_sb[kt][:, 0:128],
                        rhs=xT_sb[kt][:, ts(n, 512)], **st,
                    )
                for n in range(2):
                    nc.tensor.matmul(
                        qk3[n][:], lhsT=wq_sb[kt][:, 0:128],
                        rhs=xT_sb[kt][:, ts(2 + n, 512)], **st,
                    )

            def k_mms(kt):
                st = dict(start=(kt == 0), stop=(kt == NKT - 1))
                for n in range(4):
                    nc.tensor.matmul(
                        kpv[n][:], lhsT=wk_sb[kt][:, 0:128],
                        rhs=xT_sb[kt][:, ts(n, 512)], **st,
                    )

            for kt in range(NKT):
                q_mms(kt)
                if kt >= 1:
                    k_mms(kt - 1)
            k_mms(NKT - 1)
            # evac: k on DVE (frees attention slots first), q on ACT
            for n in range(4):
                nc.vector.tensor_copy(kT_sb[0][:, ts(n, 512)], kpv[n][:])
            for n in range(2):
                nc.scalar.copy(qT_sb[0][:, ts(n, 512)], qa[:, ts(n, 512)])
            for n in range(2):
                nc.vector.tensor_copy(qT_sb[0][:, ts(2 + n, 512)], qk3[n][:])

            # ================= attention helpers =================
            def scores_exp_mask(mp, kt, e_tiles):
                nkt = 128 * min(3, NT - kt)
                sc = psA.tile([128, 1024], F32, tag="big", name="ps_sc")
                hs = 512
                for half in range(2):
                    rows = slice(64 * half, 64 * half + 64)
                    nc.tensor.matmul(
                        sc[:, ds(hs * half, nkt)],
                        lhsT=kT_sb[mp][rows, ts(kt, 128)],
                        rhs=qT_sb[mp][rows, ds(128 * kt, nkt)],
                        start=True, stop=True,
                    )
                E = ep.tile([128, 768], BF, tag="E", name="E")
                scv = sc.rearrange("p (b x) -> p b x", b=2)
                Ev = E.rearrange("p (b x) -> p b x", b=2)
                nc.scalar.activation(
                    Ev[:, :, 0:nkt], scv[:, :, 0:nkt],
                    mybir.ActivationFunctionType.Exp, scale=SCALE,
                )
                # fused mask: diag (cols 0:128) + strict (cols 256:384)
                if kt <= NT - 3:
                    Em = E.rearrange("p (b s x) -> p b s x", b=2, s=3)[:, :, ::2, :]
                    nc.vector.tensor_mul(Em[:], Em[:], maskf_v[:])
                else:
                    nc.vector.tensor_mul(Ev[:, :, 0:128], Ev[:, :, 0:128], maskd_v[:])
                e_tiles.append(E)

            def pv_step(mp, h, j, e_tiles, pvps):
                """PV accumulation for head h (0/1 within pair), query tile j."""
                if j % 4 == 0:
                    pvps[h] = psB.tile([128, QB], F32, tag="pv", bufs=4, name="ps_pv")
                ps = pvps[h]
                col = 128 * (j % 4)
                kts = [k2 for k2 in (j - 2, j - 1, j) if k2 >= 0]
                for idx, k2 in enumerate(kts):
                    nc.tensor.matmul(
                        ps[0:65, ds(col, 128)],
                        lhsT=v_sb[k2][:, ds(65 * (2 * mp + h), 65)],
                        rhs=e_tiles[k2][:, ds(384 * h + 128 * (j - k2), 128)],
                        start=(idx == 0), stop=(idx == len(kts) - 1),
                    )

            def norm_recip_half(pvps, pend, b, half):
                """last-blk tail: normalize one 256-col half early."""
                units = []
                co = 256 * half
                for h in range(2):
                    ps = pvps[h]
                    d_sb = smallp.tile([1, 256], F32, tag="dh", name="d_half")
                    nc.scalar.copy(d_sb[:], ps[64:65, ds(co, 256)])
                    r = smallp.tile([1, 256], F32, tag="rh", name="r_half")
                    nc.vector.reciprocal_approx_fast(r[:], d_sb[:])
                    r_dr = dramp.tile([1, 256], F32, tag="rdrh", name="r_drh")
                    nc.gpsimd.dma_start(r_dr[:], r[:])
                    rb = smallp.tile([64, 256], F32, tag="rbh", name="rb_half")
                    nc.gpsimd.dma_start(rb[:], r_dr[:].to_broadcast([64, 256]))
                    units.append((h, ps, rb))
                pend[(b, half)] = units

            def norm_flush_half(mp, pend, b, half):
                co = 256 * half
                for h, ps, rb in pend.pop((b, half)):
                    nc.vector.tensor_mul(
                        aT_sb[mp][ds(64 * h, 64), ds(QB * b + co, 256)],
                        ps[0:64, ds(co, 256)], rb[:],
                    )

            def norm_recip(pvps, pend, b):
                """blk b complete: 1/D on DVE (from the PSUM ones-row), then
                partition-broadcast via DRAM bounce.  mul deferred to hide
                the DMA latency."""
                units = []
                for h in range(2):
                    ps = pvps[h]
                    r = smallp.tile([1, QB], F32, tag="r", name="r_row")
                    if STAGE_RECIP:
                        d_sb = smallp.tile([1, QB], F32, tag="d", name="d_row")
                        nc.scalar.copy(d_sb[:], ps[64:65, :])
                        nc.vector.reciprocal_approx_fast(r[:], d_sb[:])
                    else:
                        nc.vector.reciprocal_approx_fast(r[:], ps[64:65, :])
                    r_dr = dramp.tile([1, QB], F32, tag="rdr", name="r_dr")
                    nc.gpsimd.dma_start(r_dr[:], r[:])
                    rb = smallp.tile([64, QB], F32, tag="rb", name="rb_bc")
                    nc.gpsimd.dma_start(rb[:], r_dr[:].to_broadcast([64, QB]))
                    units.append((h, ps, rb))
                pend[b] = units

            def norm_flush(mp, pend, b):
                """deferred: one DVE mul against the broadcast 1/D row."""
                for h, ps, rb in pend.pop(b):
                    nc.vector.tensor_mul(
                        aT_sb[mp][ds(64 * h, 64), ds(QB * b, QB)],
                        ps[0:64, :], rb[:],
                    )

            def v_tile(t):
                pv = psB.tile([128, CL], F32, tag="k3", bufs=2, name="ps_v")
                for kt in range(NKT):
                    nc.tensor.matmul(
                        pv[:], lhsT=xT_sb[kt][:, ts(t, 128)], rhs=wv_sb[kt][:],
                        start=(kt == 0), stop=(kt == NKT - 1),
                    )
                vvv = v_sb[t].rearrange("p (h c) -> p h c", h=4)
                if t % 2 == 0:
                    nc.vector.tensor_copy(
                        vvv[:, :, 0:64], pv.rearrange("p (h c) -> p h c", h=4)[:]
                    )
                else:
                    nc.scalar.activation(
                        vvv[:, :, 0:64], pv.rearrange("p (h c) -> p h c", h=4)[:],
                        mybir.ActivationFunctionType.Copy,
                    )

            def outproj_tile(t, use_big=False):
                """y tile t: two 512-wide halves in the k3 ring (or one big
                slot for late tiles, halving evac pressure on each ring)."""
                ysb = yp.tile([128, C], BF, tag="y", name="ysb")
                if use_big:
                    ps = psA.tile([128, 1024], F32, tag="big", name="ps_yb")
                    pss = [ps[:, ts(0, 512)], ps[:, ts(1, 512)]]
                else:
                    pss = [psB.tile([128, 512], F32, tag="k3", bufs=2, name="ps_y")[:]
                           for _ in range(2)]
                for kj in range(2):
                    for n in range(2):
                        nc.tensor.matmul(
                            pss[n], lhsT=aT_sb[kj][:, ts(t, 128)],
                            rhs=wo_sb[kj][:, ts(n, 512)],
                            start=(kj == 0), stop=(kj == 1),
                        )
                nc.scalar.copy(ysb[:, ts(0, 512)], pss[0])
                nc.vector.tensor_copy(ysb[:, ts(1, 512)], pss[1])
                nc.sync.dma_start(y_d[ts(t, 128), :], ysb[:])

            # ================= phase B: v-proj + pair-0 attention =================
            e0, pvps0, pend0 = [], {}, {}
            for t in range(NT + 4):
                j = t - 2
                # deferred bcast+mul for blk (j-5)//4 first (frees pv ring)
                if j >= 5 and j % 4 == 1:
                    norm_flush(0, pend0, (j - 5) // 4)
                if t < NT:
                    scores_exp_mask(0, t, e0)
                if 0 <= j < NT:
                    pv_step(0, 0, j, e0, pvps0)
                    pv_step(0, 1, j, e0, pvps0)
                    if j % 4 == 3:
                        norm_recip(pvps0, pend0, j // 4)
                if t < NT:
                    v_tile(t)

            # ====== phases C+D merged: m1 qk chunks + pair-1 attention +
            # out-projection, one software-pipelined stream.  Chunks fill the
            # first 8 iterations and cover the score-drain exp chains. ======
            e1, pvps1, pend1 = [], {}, {}
            sc1_done = 0

            def drain_sc1(upto):
                nonlocal sc1_done
                while sc1_done < min(upto, NT):
                    scores_exp_mask(1, sc1_done, e1)
                    sc1_done += 1

            for t in range(2, NT + 9):
                c = t - 2
                if c < 8:
                    n, proj = c // 2, c % 2
                    w_sbl, dstT = ((wq_sb, qT_sb), (wk_sb, kT_sb))[proj]
                    ps = psB.tile([128, 512], F32, tag="k3", bufs=2, name="ps_m1")
                    for kt in range(NKT):
                        nc.tensor.matmul(
                            ps[:], lhsT=w_sbl[kt][:, ds(128, 128)],
                            rhs=xT_sb[kt][:, ts(n, 512)],
                            start=(kt == 0), stop=(kt == NKT - 1),
                        )
                    nc.vector.tensor_copy(dstT[1][:, ts(n, 512)], ps[:])
                j = t - 4
                if j >= 6 and j % 4 == 2 and (j - 6) // 4 < NBLK:
                    b = (j - 6) // 4
                    norm_flush(1, pend1, b)
                    for ot in range(4 * b, 4 * b + 4):
                        outproj_tile(ot, use_big=(b >= 2 and ot % 2 == 1))
                # chunk c evacs at iteration end; drains trail by one chunk
                if t >= 4:
                    drain_sc1(min(sc1_done + 2, 2 * (t - 3)))
                if 0 <= j < NT:
                    pv_step(1, 0, j, e1, pvps1)
                    pv_step(1, 1, j, e1, pvps1)
                    if j % 4 == 3:
                        norm_recip(pvps1, pend1, j // 4)

    nc.compile()
    return nc


def make_masks():
    one = np.ones((128, 128), np.float32)
    maskd = np.triu(one)          # keep iff i >= kk  (diag tile)
    masks_ = np.tril(one, -1)     # keep iff i <  kk  (strict tile)
    md2 = np.concatenate([maskd, maskd], axis=1).astype(BF16)
    ms2 = np.concatenate([masks_, masks_], axis=1).astype(BF16)
    return md2, ms2


def make_in_maps(x, wq, wk, wv, wo):
    x = np.asarray(x, np.float32)
    wq, wk, wv, wo = (np.asarray(a, np.float32) for a in (wq, wk, wv, wo))
    md2, ms2 = make_masks()
    md, ms = md2[:, :128], ms2[:, :128]
    mask_all = np.hstack([md, ms, md, ms])  # [128, 512]

    def tile_rows(a):  # [1024, W] -> [128, 8*W] (row-blocks side by side)
        return np.hstack([a[i * 128 : (i + 1) * 128] for i in range(a.shape[0] // 128)])

    in_maps = []
    xTts = [tile_rows(np.ascontiguousarray(x[b].T).astype(BF16)) for b in range(B)]
    for c in range(8):
        b, g = divmod(c, 4)
        sl = slice(g * CL, (g + 1) * CL)
        wt = np.hstack(
            [
                tile_rows(np.ascontiguousarray(w[sl, :].T).astype(BF16))
                for w in (wq, wk, wv)
            ]
        )
        wot = tile_rows(np.ascontiguousarray(wo[:, sl].T).astype(BF16))
        in_maps.append(
            {"xTt": xTts[b], "wt": wt, "wot": wot, "maskt": mask_all}
        )
    return in_maps


_PROG = None


def _get_prog():
    global _PROG
    if _PROG is None:
        _PROG = build_program()
    return _PROG


def kernel(x, wq, wk, wv, wo, _trace=False, _tmpdir=None):
    from concourse.bass_utils import run_bass_kernel_spmd

    nc = _get_prog()
    in_maps = make_in_maps(x, wq, wk, wv, wo)
    res = run_bass_kernel_spmd(
        nc, in_maps, core_ids=list(range(8)), trace=_trace, tmpdir=_tmpdir
    )
    y = np.zeros((B, T, C), np.float32)
    for c in range(8):
        b = c // 4
        y[b] += res.results[c]["y"].astype(np.float32)
    if _trace:
        kernel._last_results = res
    return y


# revision 23
# speedup vs baseline: 1.1501x; 1.1501x over previous
"""Sliding-window causal attention (B=2,T=2048,C=1024,H=16,D=64,W=256) on 8 trn2 cores.

Sharding: core c = (batch b = c//4, head-group g = c%4 of 4 heads).
Each core computes q/k/v projections for its 4 heads on its batch, windowed
attention per head, and a partial output projection (its 256 channels of the
contraction); host sums the 4 partials per batch.

v2 pipeline (vs v1 baseline):
  - q/k (head-pair 0) projections accumulate kt-major so matmuls track the
    streaming xT DMA instead of waiting for the full 4MB.
  - v-projection is interleaved with pair-0 attention; pair-1 qk projection
    is interleaved with pair-1 scores; out-projection streams behind pair-1
    PV normalization.  PE never idles long enough to re-throttle (HAM).
  - softmax normalization: ACT stages the ones-row denominator to SBUF
    (custom-DVE recip misreads PSUM on HW), DVE reciprocal, partition-
    broadcast via DRAM bounce DMA, one DVE multiply, all deferred half a
    block to hide the latency.
  - PSUM (8 banks): big 1x[128,1024] (scores, 1-deep) | pv 4x[128,512]
    (k-accs, PV+broadcast) | k3 2x[128,512] (warmup, q-accs, v, m1, outproj).
"""

import os
import sys

sys.path.insert(0, "/opt/trn_rl_repo")

import numpy as np
import ml_dtypes

import concourse.bass as bass
import concourse.tile as tile
from concourse import bacc
from concourse import mybir
from concourse.bass import ds, ts

BF16 = ml_dtypes.bfloat16

B, T, C = 2, 2048, 1024
H, W, D = 16, 256, 64
HPC = 4          # heads per core
CL = HPC * D     # 256 local channels per core
NKT = C // 128   # 8 contraction tiles for projections
NT = T // 128    # 16 token tiles
SCALE = 0.125    # 1/sqrt(D)
F32 = mybir.dt.float32
BF = mybir.dt.bfloat16
QB = 512         # normalization granularity along q (4 query tiles)
NBLK = T // QB   # 4 norm blocks
# use an ACT stage copy for the denominator instead of DVE-recip from PSUM
STAGE_RECIP = os.environ.get("ATTN_STAGE_RECIP", "1") == "1"


def build_program():
    nc = bacc.Bacc("TRN2", target_bir_lowering=False, debug=False)

    xT_d = nc.dram_tensor("xTt", [128, NKT * T], BF, kind="ExternalInput")
    w_d = nc.dram_tensor("wt", [128, 3 * NKT * CL], BF, kind="ExternalInput")
    wo_d = nc.dram_tensor("wot", [128, 2 * C], BF, kind="ExternalInput")
    mask_d = nc.dram_tensor("maskt", [128, 512], BF, kind="ExternalInput")
    y_d = nc.dram_tensor("y", [T, C], BF, kind="ExternalOutput")

    with tile.TileContext(nc) as tc:
        with (
            tc.tile_pool(name="const", bufs=1) as constp,
            tc.tile_pool(name="acts", bufs=1) as actsp,
            tc.tile_pool(name="epool", bufs=18) as ep,
            tc.tile_pool(name="small", bufs=4) as smallp,
            tc.tile_pool(name="ysb", bufs=4) as yp,
            tc.tile_pool(name="psA", bufs=1, space="PSUM") as psA,
            tc.tile_pool(name="psB", bufs=1, space="PSUM") as psB,
            tc.tile_pool(name="dramp", bufs=4, space="DRAM") as dramp,
        ):
            # ---- DMA order: mask (warmup), wq+wk (kt-major proj), xT in
            # kt-chunks (512KB each), then wv, wo ----
            mask_all = constp.tile([128, 512], BF, tag="maskall", name="mask_all")
            nc.sync.dma_start(mask_all[:], mask_d[:])
            w_all = constp.tile([128, 3 * NKT * CL], BF, tag="wall", name="w_all")
            xT_all = constp.tile([128, NKT * T], BF, tag="xTall", name="xT_all")
            wo_all = constp.tile([128, 2 * C], BF, tag="woall", name="wo_all")

            def w_load(wi):
                nc.sync.dma_start(
                    w_all[:, ds(wi * NKT * CL, NKT * CL)],
                    w_d[:, ds(wi * NKT * CL, NKT * CL)],
                )

            def x_load(kt):
                nc.sync.dma_start(xT_all[:, ds(kt * T, T)], xT_d[:, ds(kt * T, T)])

            x_load(0)
            w_load(0)          # wq
            w_load(1)          # wk
            for kt in range(1, 7):
                x_load(kt)
            w_load(2)          # wv
            x_load(7)
            nc.sync.dma_start(wo_all[:], wo_d[:])

            xT_sb = [xT_all[:, ds(i * T, T)] for i in range(NKT)]
            wq_sb = [w_all[:, ds((0 * NKT + i) * CL, CL)] for i in range(NKT)]
            wk_sb = [w_all[:, ds((1 * NKT + i) * CL, CL)] for i in range(NKT)]
            wv_sb = [w_all[:, ds((2 * NKT + i) * CL, CL)] for i in range(NKT)]
            wo_sb = [wo_all[:, ds(j * C, C)] for j in range(2)]
            maskf_v = mask_all.rearrange("p (b s x) -> p b s x", b=2, s=2)
            maskd_v = maskf_v[:, :, 0, :]

            ones_sb = constp.tile([1, 64], BF, tag="ones", name="ones_sb")
            nc.gpsimd.memset(ones_sb[:], 1.0)

            # persistent activations
            qT_sb = [actsp.tile([128, T], BF, tag=f"qT{m}", name=f"qT{m}") for m in range(2)]
            kT_sb = [actsp.tile([128, T], BF, tag=f"kT{m}", name=f"kT{m}") for m in range(2)]
            aT_sb = [actsp.tile([128, T], BF, tag=f"aT{m}", name=f"aT{m}") for m in range(2)]
            # v per token-tile: [v_h0(64)|1|v_h1(64)|1|...] = 260 cols
            v_sb = [actsp.tile([128, 4 * 65], BF, tag=f"v{t}", name=f"v{t}") for t in range(NT)]
            for t in range(NT):
                vv = v_sb[t].rearrange("p (h c) -> p h c", h=4)
                nc.gpsimd.memset(vv[:, :, 64:65], 1.0)

            # ---- PE warmup during initial DMA (HAM to 2.4GHz) ----
            n_warm = int(os.environ.get("ATTN_WARMUP_MMS", "24"))
            if n_warm:
                wps = psB.tile([128, 512], F32, tag="k3", bufs=2, name="ps_warm")
                for _ in range(n_warm):
                    nc.tensor.matmul(
                        wps[:], lhsT=mask_all[:, 0:128], rhs=mask_all[:],
                        start=True, stop=True,
                    )

            # ================= phase A: q/k head-pair 0, kt-major =================
            # q chunks 0-1 in the big slot, 2-3 in k3 slots; k chunks in pv slots
            qa = psA.tile([128, 1024], F32, tag="big", name="qa")
            qk3 = [psB.tile([128, 512], F32, tag="k3", bufs=2, name=f"qk3{i}") for i in range(2)]
            kpv = [psB.tile([128, 512], F32, tag="pv", bufs=4, name=f"kpv{i}") for i in range(4)]

            def q_mms(kt):
                st = dict(start=(kt == 0), stop=(kt == NKT - 1))
                for n in range(2):
                    nc.tensor.matmul(
                        qa[:, ts(n, 512)], lhsT=wq_sb[kt][:, 0:128],
                        rhs=xT_sb[kt][:, ts(n, 512)], **st,
                    )
                for n in range(2):
                    nc.tensor.matmul(
                        qk3[n][:], lhsT=wq_sb[kt][:, 0:128],
                        rhs=xT_sb[kt][:, ts(2 + n, 512)], **st,
                    )

            def k_mms(kt):
                st = dict(start=(kt == 0), stop=(kt == NKT - 1))
                for n in range(4):
                    nc.tensor.matmul(
                        kpv[n][:], lhsT=wk_sb[kt][:, 0:128],
                        rhs=xT_sb[kt][:, ts(n, 512)], **st,
                    )

            for kt in range(NKT):
                q_mms(kt)
                if kt >= 1:
                    k_mms(kt - 1)
            k_mms(NKT - 1)
            # evac: k on DVE (frees attention slots first), q on ACT
            for n in range(4):
                nc.vector.tensor_copy(kT_sb[0][:, ts(n, 512)], kpv[n][:])
            for n in range(2):
                nc.scalar.copy(qT_sb[0][:, ts(n, 512)], qa[:, ts(n, 512)])
            for n in range(2):
                nc.vector.tensor_copy(qT_sb[0][:, ts(2 + n, 512)], qk3[n][:])

            # ================= attention helpers =================
            def scores_exp_mask(mp, kt, e_tiles):
                nkt = 128 * min(3, NT - kt)
                sc = psA.tile([128, 1024], F32, tag="big", name="ps_sc")
                hs = 512
                for half in range(2):
                    rows = slice(64 * half, 64 * half + 64)
                    nc.tensor.matmul(
                        sc[:, ds(hs * half, nkt)],
                        lhsT=kT_sb[mp][rows, ts(kt, 128)],
                        rhs=qT_sb[mp][rows, ds(128 * kt, nkt)],
                        start=True, stop=True,
                    )
                E = ep.tile([128, 768], BF, tag="E", name="E")
                scv = sc.rearrange("p (b x) -> p b x", b=2)
                Ev = E.rearrange("p (b x) -> p b x", b=2)
                nc.scalar.activation(
                    Ev[:, :, 0:nkt], scv[:, :, 0:nkt],
                    mybir.ActivationFunctionType.Exp, scale=SCALE,
                )
                # fused mask: diag (cols 0:128) + strict (cols 256:384)
                if kt <= NT - 3:
                    Em = E.rearrange("p (b s x) -> p b s x", b=2, s=3)[:, :, ::2, :]
                    nc.vector.tensor_mul(Em[:], Em[:], maskf_v[:])
                else:
                    nc.vector.tensor_mul(Ev[:, :, 0:128], Ev[:, :, 0:128], maskd_v[:])
                e_tiles.append(E)

            def pv_step(mp, h, j, e_tiles, pvps):
                """PV accumulation for head h (0/1 within pair), query tile j."""
                if j % 4 == 0:
                    pvps[h] = psB.tile([128, QB], F32, tag="pv", bufs=4, name="ps_pv")
                ps = pvps[h]
                col = 128 * (j % 4)
                kts = [k2 for k2 in (j - 2, j - 1, j) if k2 >= 0]
                for idx, k2 in enumerate(kts):
                    nc.tensor.matmul(
                        ps[0:65, ds(col, 128)],
                        lhsT=v_sb[k2][:, ds(65 * (2 * mp + h), 65)],
                        rhs=e_tiles[k2][:, ds(384 * h + 128 * (j - k2), 128)],
                        start=(idx == 0), stop=(idx == len(kts) - 1),
                    )

            def norm_recip(pvps, pend, b):
                """blk b complete: 1/D on DVE (from the PSUM ones-row), then
                partition-broadcast via DRAM bounce.  mul deferred to hide
                the DMA latency."""
                units = []
                for h in range(2):
                    ps = pvps[h]
                    r = smallp.tile([1, QB], F32, tag="r", name="r_row")
                    if STAGE_RECIP:
                        d_sb = smallp.tile([1, QB], F32, tag="d", name="d_row")
                        nc.scalar.copy(d_sb[:], ps[64:65, :])
                        nc.vector.reciprocal_approx_fast(r[:], d_sb[:])
                    else:
                        nc.vector.reciprocal_approx_fast(r[:], ps[64:65, :])
                    r_dr = dramp.tile([1, QB], F32, tag="rdr", name="r_dr")
                    nc.gpsimd.dma_start(r_dr[:], r[:])
                    rb = smallp.tile([64, QB], F32, tag="rb", name="rb_bc")
                    nc.gpsimd.dma_start(rb[:], r_dr[:].to_broadcast([64, QB]))
                    units.append((h, ps, rb))
                pend[b] = units

            def norm_flush(mp, pend, b):
                """deferred: one DVE mul against the broadcast 1/D row."""
                for h, ps, rb in pend.pop(b):
                    nc.vector.tensor_mul(
                        aT_sb[mp][ds(64 * h, 64), ds(QB * b, QB)],
                        ps[0:64, :], rb[:],
                    )

            def v_tile(t):
                pv = psB.tile([128, CL], F32, tag="k3", bufs=2, name="ps_v")
                for kt in range(NKT):
                    nc.tensor.matmul(
                        pv[:], lhsT=xT_sb[kt][:, ts(t, 128)], rhs=wv_sb[kt][:],
                        start=(kt == 0), stop=(kt == NKT - 1),
                    )
                vvv = v_sb[t].rearrange("p (h c) -> p h c", h=4)
                if t % 2 == 0:
                    nc.vector.tensor_copy(
                        vvv[:, :, 0:64], pv.rearrange("p (h c) -> p h c", h=4)[:]
                    )
                else:
                    nc.scalar.activation(
                        vvv[:, :, 0:64], pv.rearrange("p (h c) -> p h c", h=4)[:],
                        mybir.ActivationFunctionType.Copy,
                    )

            def outproj_tile(t, use_big=False):
                """y tile t: two 512-wide halves in the k3 ring (or one big
                slot for late tiles, halving evac pressure on each ring)."""
                ysb = yp.tile([128, C], BF, tag="y", name="ysb")
                if use_big:
                    ps = psA.tile([128, 1024], F32, tag="big", name="ps_yb")
                    pss = [ps[:, ts(0, 512)], ps[:, ts(1, 512)]]
                else:
                    pss = [psB.tile([128, 512], F32, tag="k3", bufs=2, name="ps_y")[:]
                           for _ in range(2)]
                for kj in range(2):
                    for n in range(2):
                        nc.tensor.matmul(
                            pss[n], lhsT=aT_sb[kj][:, ts(t, 128)],
                            rhs=wo_sb[kj][:, ts(n, 512)],
                            start=(kj == 0), stop=(kj == 1),
                        )
                nc.scalar.copy(ysb[:, ts(0, 512)], pss[0])
                nc.vector.tensor_copy(ysb[:, ts(1, 512)], pss[1])
                nc.sync.dma_start(y_d[ts(t, 128), :], ysb[:])

            # ================= phase B: v-proj + pair-0 attention =================
            e0, pvps0, pend0 = [], {}, {}
            for t in range(NT + 4):
                j = t - 2
                # deferred bcast+mul for blk (j-5)//4 first (frees pv ring)
                if j >= 5 and j % 4 == 1:
                    norm_flush(0, pend0, (j - 5) // 4)
                if t < NT:
                    scores_exp_mask(0, t, e0)
                if 0 <= j < NT:
                    pv_step(0, 0, j, e0, pvps0)
                    pv_step(0, 1, j, e0, pvps0)
                    if j % 4 == 3:
                        norm_recip(pvps0, pend0, j // 4)
                if t < NT:
                    v_tile(t)

            # ====== phases C+D merged: m1 qk chunks + pair-1 attention +
            # out-projection, one software-pipelined stream.  Chunks fill the
            # first 8 iterations and cover the score-drain exp chains. ======
            e1, pvps1, pend1 = [], {}, {}
            sc1_done = 0

            def drain_sc1(upto):
                nonlocal sc1_done
                while sc1_done < min(upto, NT):
                    scores_exp_mask(1, sc1_done, e1)
                    sc1_done += 1

            for t in range(2, NT + 9):
                c = t - 2
                if c < 8:
                    n, proj = c // 2, c % 2
                    w_sbl, dstT = ((wq_sb, qT_sb), (wk_sb, kT_sb))[proj]
                    ps = psB.tile([128, 512], F32, tag="k3", bufs=2, name="ps_m1")
                    for kt in range(NKT):
                        nc.tensor.matmul(
                            ps[:], lhsT=w_sbl[kt][:, ds(128, 128)],
                            rhs=xT_sb[kt][:, ts(n, 512)],
                            start=(kt == 0), stop=(kt == NKT - 1),
                        )
                    nc.vector.tensor_copy(dstT[1][:, ts(n, 512)], ps[:])
                j = t - 4
                if j >= 6 and j % 4 == 2 and (j - 6) // 4 < NBLK:
                    b = (j - 6) // 4
                    norm_flush(1, pend1, b)
                    for ot in range(4 * b, 4 * b + 4):
                        outproj_tile(ot, use_big=(b >= 2 and ot % 2 == 1))
                # chunk c evacs at iteration end; drains trail by one chunk
                if t >= 4:
                    drain_sc1(min(sc1_done + 2, 2 * (t - 3)))
                if 0 <= j < NT:
                    pv_step(1, 0, j, e1, pvps1)
                    pv_step(1, 1, j, e1, pvps1)
                    if j % 4 == 3:
                        norm_recip(pvps1, pend1, j // 4)

    nc.compile()
    return nc


def make_masks():
    one = np.ones((128, 128), np.float32)
    maskd = np.triu(one)          # keep iff i >= kk  (diag tile)
    masks_ = np.tril(one, -1)     # keep iff i <  kk  (strict tile)
    md2 = np.concatenate([maskd, maskd], axis=1).astype(BF16)
    ms2 = np.concatenate([masks_, masks_], axis=1).astype(BF16)
    return md2, ms2


def make_in_maps(x, wq, wk, wv, wo):
    x = np.asarray(x, np.float32)
    wq, wk, wv, wo = (np.asarray(a, np.float32) for a in (wq, wk, wv, wo))
    md2, ms2 = make_masks()
    md, ms = md2[:, :128], ms2[:, :128]
    mask_all = np.hstack([md, ms, md, ms])  # [128, 512]

    def tile_rows(a):  # [1024, W] -> [128, 8*W] (row-blocks side by side)
        return np.hstack([a[i * 128 : (i + 1) * 128] for i in range(a.shape[0] // 128)])

    in_maps = []
    xTts = [tile_rows(np.ascontiguousarray(x[b].T).astype(BF16)) for b in range(B)]
    for c in range(8):
        b, g = divmod(c, 4)
        sl = slice(g * CL, (g + 1) * CL)
        wt = np.hstack(
            [
                tile_rows(np.ascontiguousarray(w[sl, :].T).astype(BF16))
                for w in (wq, wk, wv)
            ]
        )
        wot = tile_rows(np.ascontiguousarray(wo[:, sl].T).astype(BF16))
        in_maps.append(
            {"xTt": xTts[b], "wt": wt, "wot": wot, "maskt": mask_all}
        )
    return in_maps


_PROG = None


def _get_prog():
    global _PROG
    if _PROG is None:
        _PROG = build_program()
    return _PROG


def kernel(x, wq, wk, wv, wo, _trace=False, _tmpdir=None):
    from concourse.bass_utils import run_bass_kernel_spmd

    nc = _get_prog()
    in_maps = make_in_maps(x, wq, wk, wv, wo)
    res = run_bass_kernel_spmd(
        nc, in_maps, core_ids=list(range(8)), trace=_trace, tmpdir=_tmpdir
    )
    y = np.zeros((B, T, C), np.float32)
    for c in range(8):
        b = c // 4
        y[b] += res.results[c]["y"].astype(np.float32)
    if _trace:
        kernel._last_results = res
    return y
